# revision 16
# baseline (speedup 1.0000x reference)
"""CTC-style forward-algorithm loss kernel for Trainium2 (8 NeuronCores, data parallel).

Per core (32 batches): the reference DP
    fwd_t[s] = logaddexp(fwd_{t-1}[s] + xt4, fwd_{t-1}[s-1] + xt[k_{s-1}])
runs in probability space column-by-column over s.  The t axis (2000) splits
into NC=4 chunks on partition groups (4 chunks x 32 batches = 128 partitions),
wavefront with lag 2: at step sigma, chunk c scans column s = sigma - 2c.
Per column the DVE does ONE tensor_tensor multiply (u = esel' * prv) and ONE
affine scan (cur_t = d0*cur_{t-1} + u_t).

The transition weights esel[t] = exp(x[t, seqs[s-1]] - x[t, 4]) are a pure
function of the inputs: precomputed on the host in wavefront order, streamed
in per column over the DMA queues.

Numerics: a per-partition drain rate beta (adapted every GRP columns from the
measured column growth lz) keeps values O(1) along t; per-column
renormalization is applied to the *incoming* esel tile on the Scalar engine
(scale by 1/z of the column finished two steps earlier), keeping the
renormalization off the DVE critical path.  Log bookkeeping (PN/lt chains,
all [128,1] ops on GPSIMD) recovers exact log values; chunk halos transfer
through a PE permutation matmul (+32 partition shift) and one Scalar exp,
pipelined one column ahead.  Output is the lt chain of the last chunk group.
"""

import numpy as np

import concourse.bass as bass
import concourse.tile as tile
import concourse.mybir as mybir
from concourse.bass_utils import run_bass_kernel_spmd

NT, NB, NF, NS = 2000, 256, 5, 500
M = 8                 # cores
B = NB // M           # 32 local batches
NC = 4                # t-chunk groups on partitions
TC = NT // NC         # 500
GRP = 16              # beta update period (columns)
LAG = 2               # wavefront lag between chunk groups
SIGB = NS + LAG * (NC - 1) + 1   # 507 scan steps (sigma = 0..506)
ZFLOOR = 1e-30
F32 = mybir.dt.float32
AL = mybir.AluOpType
AF = mybir.ActivationFunctionType

_CACHE = {}


def _split_multi_waits(nc, max_inline=1):
    """walrus codegen allows only a tiny number of fused sem-waits per TPB
    instruction (Tile's native encoder is more permissive).  Hoist excess
    waits onto InstEventSemaphore carriers on the same engine stream."""
    f = nc.m.functions[0]
    n_split = 0
    for bb in f.blocks:
        out = []
        changed = False
        for inst in bb.instructions:
            si = inst.sync_info
            waits = list(si.on_wait) if si is not None and si.on_wait else []
            if isinstance(inst, mybir.InstEventSemaphore) and len(waits) > 2:
                keep, extra = waits[:2], waits[2:]
            elif not isinstance(inst, mybir.InstEventSemaphore) \
                    and len(waits) > max_inline:
                keep, extra = waits[:max_inline], waits[max_inline:]
            else:
                keep, extra = waits, []
            if extra:
                # EventSemaphore carriers hold <= 2 waits each (walrus limit)
                for j in range(0, len(extra), 2):
                    evt = mybir.InstEventSemaphore(
                        name=f"evw{j}_{inst.name}", ins=[], outs=[])
                    evt.engine = inst.engine
                    evt.sync_info = mybir.SyncInfo(
                        on_wait=extra[j:j + 2], on_update=[])
                    out.append(evt)
                inst.sync_info = mybir.SyncInfo(
                    on_wait=keep, on_update=list(si.on_update))
                changed = True
                n_split += 1
            out.append(inst)
        if changed:
            bb.instructions = out
    return n_split


def build_module(split_waits=True):
    key = ("nc", split_waits)
    if key in _CACHE:
        return _CACHE[key], _CACHE["names"]
    nc = bass.Bass(debug=False)
    v, g, s = nc.vector, nc.gpsimd, nc.scalar

    esel_d = nc.dram_tensor("esel", [128, SIGB * TC], F32, kind="ExternalInput")
    iot_d = nc.dram_tensor("iot", [128, 512], F32, kind="ExternalInput")
    perm_d = nc.dram_tensor("perm", [128, 128], F32, kind="ExternalInput")
    lout_d = nc.dram_tensor("lout", [32, 512], F32, kind="ExternalOutput")

    with tile.TileContext(nc) as tc, \
         tc.tile_pool(name="state", bufs=1) as st:
        iot = st.tile([128, 512], F32, tag="iot_sb", name="iot_sb")
        perm = st.tile([128, 128], F32, tag="perm_sb", name="perm_sb")
        bufA = st.tile([128, TC + 1], F32, tag="bufA", name="bufA")
        bufB = st.tile([128, TC + 1], F32, tag="bufB", name="bufB")
        d0row = st.tile([128, TC], F32, tag="d0row", name="d0row")
        ramp = st.tile([128, TC + 1], F32, tag="ramp", name="ramp")
        loutT = st.tile([128, 512], F32, tag="loutT", name="loutT")
        halT = [st.tile([128, 1], F32, tag=f"hal{i}", name=f"hal{i}")
                for i in range(2)]
        # per-partition scalars
        names = ["PN", "b500", "beta", "nbeta", "zf", "rz", "lz", "lt",
                 "tb", "bmp", "dbc", "dbcTC", "dneg", "edb", "hs", "b3",
                 "mnb", "b2", "t1", "t2"]
        sc = {n: st.tile([128, 1], F32, tag=n, name=n) for n in names}
        # constants
        fl30 = st.tile([128, 1], F32, tag="fl30", name="fl30")
        c40 = st.tile([128, 1], F32, tag="c40", name="c40")
        cm40 = st.tile([128, 1], F32, tag="cm40", name="cm40")
        rTC = st.tile([128, 1], F32, tag="rTC", name="rTC")
        zeroT = st.tile([128, 1], F32, tag="zeroT", name="zeroT")
        c3 = st.tile([128, 1], F32, tag="c3", name="c3")

        nc.sync.dma_start(out=iot[:, :], in_=iot_d.ap()[:, :])
        nc.sync.dma_start(out=perm[:, :], in_=perm_d.ap()[:, :])

        for t in (bufA, bufB, loutT):
            v.memset(t[:, :], 0.0)
        for n in names:
            g.memset(sc[n][:, :], 0.0)
        g.memset(sc["rz"][:, :], 1.0)
        v.memset(d0row[:, :], 1.0)
        g.memset(fl30[:, :], ZFLOOR)
        g.memset(c40[:, :], 40.0)
        g.memset(cm40[:, :], -40.0)
        g.memset(rTC[:, :], 1.0 / TC)
        g.memset(zeroT[:, :], 0.0)
        g.memset(c3[:, :], 3.0)
        m1 = st.tile([128, 1], F32, tag="m1", name="m1")
        g.memset(m1[:, :], -1.0)
        g.memset(halT[0][:, :], 0.0)
        g.memset(halT[0][0:32, :], 1.0)   # group 0, column 0 seed
        g.memset(halT[1][:, :], 0.0)
        # touch DMA'd tiles once so later consumers inherit the DMA dependency
        scr = st.tile([128, 1], F32, tag="scr", name="scr")
        for tl in (iot, perm):
            v.tensor_copy(scr[:, 0:1], tl[:, 0:1])
        s.copy(scr[:, 0:1], iot[:, 0:1])

        PN, b500, beta, nbeta = sc["PN"], sc["b500"], sc["beta"], sc["nbeta"]
        zf, rz, lz, lt = sc["zf"], sc["rz"], sc["lz"], sc["lt"]
        tb, bmp, dbc, dbcTC = sc["tb"], sc["bmp"], sc["dbc"], sc["dbcTC"]
        dneg, edb, hs, b3 = sc["dneg"], sc["edb"], sc["hs"], sc["b3"]
        mnb, b2, t1, t2 = sc["mnb"], sc["b2"], sc["t1"], sc["t2"]

        PRE = 5  # DMA prefetch depth
        RENK = 1  # exact renormalization period (columns)
        est_tiles = {}
        with tc.tile_pool(name="esel", bufs=PRE + 3) as esel_pool, \
             tc.tile_pool(name="upool", bufs=3) as u_pool, \
             tc.tile_pool(name="psum", bufs=2, space="PSUM") as psum_pool:

            def dma_est(k):
                if k <= SIGB - 1 and k not in est_tiles:
                    et = esel_pool.tile([128, TC], F32, tag="esel")
                    nc.sync.dma_start(out=et[:, :],
                                      in_=esel_d.ap()[:, k * TC:(k + 1) * TC])
                    est_tiles[k] = et

            for k in range(PRE):
                dma_est(k)

            for sig in range(SIGB + 1):
                cur, prv = (bufA, bufB) if sig % 2 == 0 else (bufB, bufA)
                dma_est(sig + PRE)
                hal_c = halT[sig % 2]       # consumed by scan_sig
                hal_n = halT[(sig + 1) % 2]  # produced for scan_{sig+1}
                # ---- finalize head: measure scan_{sig-1}'s end (in `prv`)
                if sig >= 1:
                    v.tensor_scalar_max(zf[:, 0:1], prv[:, TC:TC + 1], ZFLOOR)
                    v.reciprocal(rz[:, 0:1], zf[:, 0:1])
                    s.activation(lz[:, 0:1], zf[:, 0:1], AF.Ln)
                # ---- finalize tail: log chains for scan_{sig-1}
                if sig >= 1:
                    g.tensor_add(tb[:, 0:1], lz[:, 0:1], PN[:, 0:1])
                    s.activation(lt[:, 0:1], tb[:, 0:1], AF.Identity,
                                 bias=b500[:, 0:1])
                    g.tensor_add(PN[:, 0:1], tb[:, 0:1], nbeta[:, 0:1])
                if sig == SIGB:
                    s.copy(loutT[96:128, sig - 7:sig - 6], lt[96:128, 0:1])
                    break
                # ---- beta update every GRP columns
                if sig % GRP == 0 and sig > 0 and sig < SIGB:
                    v.tensor_scalar(dbc[:, 0:1], lz[:, 0:1], 40.0, -40.0,
                                    AL.min, AL.max)
                    g.tensor_mul(dbcTC[:, 0:1], dbc[:, 0:1], rTC[:, 0:1])
                    g.tensor_add(beta[:, 0:1], beta[:, 0:1], dbcTC[:, 0:1])
                    g.tensor_add(b2[:, 0:1], b2[:, 0:1], dbcTC[:, 0:1])
                    g.tensor_add(b2[:, 0:1], b2[:, 0:1], dbcTC[:, 0:1])
                    v.tensor_sub(nbeta[:, 0:1], nbeta[:, 0:1], dbcTC[:, 0:1])
                    g.tensor_add(b500[:, 0:1], b500[:, 0:1], dbc[:, 0:1])
                    v.tensor_scalar_mul(dneg[:, 0:1], dbcTC[:, 0:1], -1.0)
                    s.activation(ramp[:, 0:TC + 1], iot[:, 0:TC + 1], AF.Exp,
                                 scale=dneg[:, 0:1])
                    v.tensor_mul(prv[:, 0:TC + 1], prv[:, 0:TC + 1],
                                 ramp[:, 0:TC + 1])
                    s.activation(d0row[:, 0:TC], iot[:, 0:TC], AF.Exp,
                                 scale=0.0, bias=nbeta[:, 0:1])
                    # hal_c was built with the old beta: scale by exp(dbc/TC)
                    s.activation(edb[:, 0:1], dbcTC[:, 0:1], AF.Exp)
                    g.tensor_mul(hal_c[:, 0:1], hal_c[:, 0:1], edb[:, 0:1])
                if sig < SIGB:
                    # ---- consume halo into scan init slot; hal was built
                    # without this column's renorm scale: fold rz into the copy
                    s.activation(cur[:, 0:1], hal_c[:, 0:1], AF.Copy,
                                 scale=rz[:, 0:1])
                    # ---- u = esel * rz * prv ; affine scan along t
                    u = u_pool.tile([128, TC], F32, tag="u")
                    if sig >= 1:
                        v.scalar_tensor_tensor(u[:, :], est_tiles[sig][:, :],
                                               rz[:, 0:1], prv[:, 0:TC],
                                               AL.mult, AL.mult)
                    else:
                        v.tensor_mul(u[:, :], est_tiles[sig][:, :], prv[:, 0:TC])
                    del est_tiles[sig]
                    v.tensor_tensor_scan(cur[:, 1:TC + 1], d0row[:, :], u[:, :],
                                         cur[:, 0:1], AL.mult, AL.add)
                # ---- produce halo for scan_{sig+1}: lt shifted +32 partitions
                if sig + 1 <= SIGB - 1 and sig >= 1:
                    ltsh = psum_pool.tile([128, 1], F32, tag="ltsh")
                    nc.tensor.matmul(ltsh[:, 0:1], perm[:, :], lt[:, 0:1],
                                     start=True, stop=True)
                    # bmp = 2*beta - PN = b2 - tb + mnb  (mnb = -nbeta, held
                    # at its pre-event value until after this block)
                    g.tensor_mul(t1[:, 0:1], tb[:, 0:1], m1[:, 0:1])
                    g.tensor_add(t2[:, 0:1], t1[:, 0:1], mnb[:, 0:1])
                    g.tensor_add(bmp[:, 0:1], t2[:, 0:1], b2[:, 0:1])
                    # clamp: finished groups' drained columns hit the z floor
                    # and their (never-consumed) halos overflow
                    v.tensor_scalar(hs[:, 0:1], ltsh[:, 0:1], bmp[:, 0:1], 85.0,
                                    AL.add, AL.min)
                    s.activation(hal_n[:, 0:1], hs[:, 0:1], AF.Exp)
                    g.memset(hal_n[0:32, 0:1], 0.0)
                    for c in range(1, NC):
                        if 2 * c > sig + 1:
                            g.memset(hal_n[32 * c:32 * c + 32, 0:1], 0.0)
                elif sig == 0:
                    g.memset(hal_n[:, 0:1], 0.0)
                if sig % GRP == 0 and sig > 0:
                    g.tensor_add(mnb[:, 0:1], mnb[:, 0:1], dbcTC[:, 0:1])
                # ---- output column (lt of group 3)
                if sig >= 1 and 0 <= sig - 7 <= NS:
                    s.copy(loutT[96:128, sig - 7:sig - 6], lt[96:128, 0:1])
                # ---- birth bookkeeping
                if sig in (1, 3, 5):
                    c = (sig + 1) // 2
                    g.memset(PN[32 * c:32 * c + 32, 0:1], 0.0)
                for c in range(1, NC):
                    if sig < 2 * c:
                        g.memset(cur[32 * c:32 * c + 32, TC:TC + 1], 1.0)

        nc.sync.dma_start(out=lout_d.ap()[:, :], in_=loutT[96:128, :])

    if split_waits:
        _split_multi_waits(nc)

    _CACHE[key] = nc
    _CACHE["names"] = dict(ins=["esel", "iot", "perm"], out="lout")
    return nc, _CACHE["names"]


def host_prep(x, seqs):
    """Build per-core input arrays. Returns list of dicts."""
    f32 = np.float32
    x8 = x.reshape(NT, M, B, NF).astype(f32, copy=False)
    sq = seqs.reshape(M, B, NS)

    iot = np.broadcast_to(np.arange(512, dtype=f32), (128, 512)).copy()
    # shift-by-32 permutation: ltsh = perm.T @ lt, ltsh[q+32] = lt[q]
    perm = np.zeros((128, 128), dtype=f32)
    for q in range(96):
        perm[q, q + 32] = 1.0

    in_maps = []
    for m in range(M):
        # esel[(c,b), sig*TC + t] = E[c*TC+t, b, seqs[b, s-1]] for
        # s = sig - 2c (1 <= s <= NS), else 0.
        xm = np.ascontiguousarray(x8[:, m].transpose(1, 2, 0))   # (B, NF, NT)
        Em = np.exp(xm[:, :4] - xm[:, 4:5], dtype=f32)           # (B, 4, NT)
        idx = sq[m][:, :, None]                                   # (B, NS, 1)
        er = np.zeros((128, SIGB * TC), dtype=f32)
        erv = er.reshape(NC, B, SIGB, TC)
        for c in range(NC):
            erv[c, :, 2 * c + 1:2 * c + 1 + NS, :] = np.take_along_axis(
                Em[:, :, c * TC:(c + 1) * TC], idx, axis=1)
        in_maps.append({"esel": er, "iot": iot, "perm": perm})
    return in_maps


def host_post(x, seqlens, louts):
    f32 = np.float32
    x8 = x.reshape(NT, M, B, NF)
    C2000 = x8[:, :, :, 4].sum(axis=0, dtype=np.float32)   # (M, B)
    lt3 = np.stack([louts[m][:, 0:NS + 1] for m in range(M)])  # (M, B, NS+1)
    fwd = lt3 + C2000[:, :, None]
    fwd = fwd.reshape(NB, NS + 1)
    out = -np.take_along_axis(fwd, seqlens[:, None].astype(np.int64), axis=1) / f32(NT)
    return out.astype(np.float32)


def kernel(x, seqs, seqlens):
    nc, names = build_module()
    in_maps = host_prep(np.asarray(x), np.asarray(seqs))
    res = run_bass_kernel_spmd(nc, in_maps, list(range(M)))
    louts = [res.results[m]["lout"] for m in range(M)]
    return host_post(np.asarray(x), np.asarray(seqlens), louts)


# revision 17
# speedup vs baseline: 1.2378x; 1.2378x over previous
"""CTC-style forward-algorithm loss kernel for Trainium2 (8 NeuronCores, data parallel).

Per core (32 batches): the reference DP
    fwd_t[s] = logaddexp(fwd_{t-1}[s] + xt4, fwd_{t-1}[s-1] + xt[k_{s-1}])
runs in probability space column-by-column over s.  The t axis (2000) splits
into NC=4 chunks on partition groups (4 chunks x 32 batches = 128 partitions),
wavefront with lag 2: at step sigma, chunk c scans column s = sigma - 2c.
Per column the DVE does ONE tensor_tensor multiply (u = esel' * prv) and ONE
affine scan (cur_t = d0*cur_{t-1} + u_t).

The transition weights esel[t] = exp(x[t, seqs[s-1]] - x[t, 4]) are a pure
function of the inputs: precomputed on the host in wavefront order, streamed
in per column over the DMA queues.

Numerics: a per-partition drain rate beta (adapted every GRP columns from the
measured column growth lz) keeps values O(1) along t; per-column
renormalization is applied to the *incoming* esel tile on the Scalar engine
(scale by 1/z of the column finished two steps earlier), keeping the
renormalization off the DVE critical path.  Log bookkeeping (PN/lt chains,
all [128,1] ops on GPSIMD) recovers exact log values; chunk halos transfer
through a PE permutation matmul (+32 partition shift) and one Scalar exp,
pipelined one column ahead.  Output is the lt chain of the last chunk group.
"""

import numpy as np

import concourse.bass as bass
import concourse.tile as tile
import concourse.mybir as mybir
from concourse.bass_utils import run_bass_kernel_spmd

NT, NB, NF, NS = 2000, 256, 5, 500
M = 8                 # cores
B = NB // M           # 32 local batches
NC = 4                # t-chunk groups on partitions
TC = NT // NC         # 500
GRP = 16              # beta update period (columns)
LAG = 2               # wavefront lag between chunk groups
SIGB = NS + LAG * (NC - 1) + 1   # 507 scan steps (sigma = 0..506)
ZFLOOR = 1e-30
F32 = mybir.dt.float32
AL = mybir.AluOpType
AF = mybir.ActivationFunctionType

_CACHE = {}


def _split_multi_waits(nc, max_inline=1):
    """walrus codegen allows only a tiny number of fused sem-waits per TPB
    instruction (Tile's native encoder is more permissive).  Hoist excess
    waits onto InstEventSemaphore carriers on the same engine stream."""
    f = nc.m.functions[0]
    n_split = 0
    for bb in f.blocks:
        out = []
        changed = False
        for inst in bb.instructions:
            si = inst.sync_info
            waits = list(si.on_wait) if si is not None and si.on_wait else []
            if isinstance(inst, mybir.InstEventSemaphore) and len(waits) > 2:
                keep, extra = waits[:2], waits[2:]
            elif not isinstance(inst, mybir.InstEventSemaphore) \
                    and len(waits) > max_inline:
                keep, extra = waits[:max_inline], waits[max_inline:]
            else:
                keep, extra = waits, []
            if extra:
                # EventSemaphore carriers hold <= 2 waits each (walrus limit)
                for j in range(0, len(extra), 2):
                    evt = mybir.InstEventSemaphore(
                        name=f"evw{j}_{inst.name}", ins=[], outs=[])
                    evt.engine = inst.engine
                    evt.sync_info = mybir.SyncInfo(
                        on_wait=extra[j:j + 2], on_update=[])
                    out.append(evt)
                inst.sync_info = mybir.SyncInfo(
                    on_wait=keep, on_update=list(si.on_update))
                changed = True
                n_split += 1
            out.append(inst)
        if changed:
            bb.instructions = out
    return n_split


def build_module(split_waits=True):
    key = ("nc", split_waits)
    if key in _CACHE:
        return _CACHE[key], _CACHE["names"]
    nc = bass.Bass(debug=False)
    v, g, s = nc.vector, nc.gpsimd, nc.scalar

    esel_d = nc.dram_tensor("esel", [128, SIGB * TC], F32, kind="ExternalInput")
    iot_d = nc.dram_tensor("iot", [128, 512], F32, kind="ExternalInput")
    perm_d = nc.dram_tensor("perm", [128, 128], F32, kind="ExternalInput")
    lout_d = nc.dram_tensor("lout", [32, 512], F32, kind="ExternalOutput")

    with tile.TileContext(nc) as tc, \
         tc.tile_pool(name="state", bufs=1) as st:
        iot = st.tile([128, 512], F32, tag="iot_sb", name="iot_sb")
        perm = st.tile([128, 128], F32, tag="perm_sb", name="perm_sb")
        bufA = st.tile([128, TC + 1], F32, tag="bufA", name="bufA")
        bufB = st.tile([128, TC + 1], F32, tag="bufB", name="bufB")
        d0row = st.tile([128, TC], F32, tag="d0row", name="d0row")
        ramp = st.tile([128, TC + 1], F32, tag="ramp", name="ramp")
        loutT = st.tile([128, 512], F32, tag="loutT", name="loutT")
        halT = [st.tile([128, 1], F32, tag=f"hal{i}", name=f"hal{i}")
                for i in range(2)]
        # per-partition scalars
        names = ["PN", "b500", "beta", "nbeta", "zf", "rz", "lz", "lt",
                 "tb", "bmp", "dbc", "dbcTC", "dneg", "edb", "hs", "b3",
                 "mnb", "b2", "t1", "t2"]
        sc = {n: st.tile([128, 1], F32, tag=n, name=n) for n in names}
        # constants
        fl30 = st.tile([128, 1], F32, tag="fl30", name="fl30")
        c40 = st.tile([128, 1], F32, tag="c40", name="c40")
        cm40 = st.tile([128, 1], F32, tag="cm40", name="cm40")
        rTC = st.tile([128, 1], F32, tag="rTC", name="rTC")
        zeroT = st.tile([128, 1], F32, tag="zeroT", name="zeroT")
        c3 = st.tile([128, 1], F32, tag="c3", name="c3")

        nc.sync.dma_start(out=iot[:, :], in_=iot_d.ap()[:, :])
        nc.sync.dma_start(out=perm[:, :], in_=perm_d.ap()[:, :])

        for t in (bufA, bufB, loutT):
            v.memset(t[:, :], 0.0)
        for n in names:
            g.memset(sc[n][:, :], 0.0)
        g.memset(sc["rz"][:, :], 1.0)
        v.memset(d0row[:, :], 1.0)
        g.memset(fl30[:, :], ZFLOOR)
        g.memset(c40[:, :], 40.0)
        g.memset(cm40[:, :], -40.0)
        g.memset(rTC[:, :], 1.0 / TC)
        g.memset(zeroT[:, :], 0.0)
        g.memset(c3[:, :], 3.0)
        m1 = st.tile([128, 1], F32, tag="m1", name="m1")
        g.memset(m1[:, :], -1.0)
        g.memset(halT[0][:, :], 0.0)
        g.memset(halT[0][0:32, :], 1.0)   # group 0, column 0 seed
        g.memset(halT[1][:, :], 0.0)
        # touch DMA'd tiles once so later consumers inherit the DMA dependency
        scr = st.tile([128, 1], F32, tag="scr", name="scr")
        for tl in (iot, perm):
            v.tensor_copy(scr[:, 0:1], tl[:, 0:1])
        s.copy(scr[:, 0:1], iot[:, 0:1])

        PN, b500, beta, nbeta = sc["PN"], sc["b500"], sc["beta"], sc["nbeta"]
        zf, rz, lz, lt = sc["zf"], sc["rz"], sc["lz"], sc["lt"]
        tb, bmp, dbc, dbcTC = sc["tb"], sc["bmp"], sc["dbc"], sc["dbcTC"]
        dneg, edb, hs, b3 = sc["dneg"], sc["edb"], sc["hs"], sc["b3"]
        mnb, b2, t1, t2 = sc["mnb"], sc["b2"], sc["t1"], sc["t2"]

        PRE = 5  # DMA prefetch depth
        RENK = 1  # exact renormalization period (columns)
        est_tiles = {}
        with tc.tile_pool(name="esel", bufs=PRE + 3) as esel_pool, \
             tc.tile_pool(name="upool", bufs=3) as u_pool, \
             tc.tile_pool(name="psum", bufs=2, space="PSUM") as psum_pool:

            def dma_est(k):
                if k <= SIGB - 1 and k not in est_tiles:
                    et = esel_pool.tile([128, TC], F32, tag="esel")
                    nc.sync.dma_start(out=et[:, :],
                                      in_=esel_d.ap()[:, k * TC:(k + 1) * TC])
                    est_tiles[k] = et

            for k in range(PRE):
                dma_est(k)

            for sig in range(SIGB + 1):
                cur, prv = (bufA, bufB) if sig % 2 == 0 else (bufB, bufA)
                dma_est(sig + PRE)
                hal_c = halT[sig % 2]       # consumed by scan_sig
                hal_n = halT[(sig + 1) % 2]  # produced for scan_{sig+1}
                # ---- finalize head: measure scan_{sig-1}'s end (in `prv`)
                if sig >= 1:
                    v.tensor_scalar_max(zf[:, 0:1], prv[:, TC:TC + 1], ZFLOOR)
                    v.reciprocal(rz[:, 0:1], zf[:, 0:1])
                    s.activation(lz[:, 0:1], zf[:, 0:1], AF.Ln)
                # ---- finalize tail: log chains for scan_{sig-1}
                if sig >= 1:
                    v.tensor_add(tb[:, 0:1], lz[:, 0:1], PN[:, 0:1])
                    s.activation(lt[:, 0:1], tb[:, 0:1], AF.Identity,
                                 bias=b500[:, 0:1])
                    v.tensor_add(PN[:, 0:1], tb[:, 0:1], nbeta[:, 0:1])
                if sig == SIGB:
                    s.copy(loutT[96:128, sig - 7:sig - 6], lt[96:128, 0:1])
                    break
                # ---- beta update every GRP columns
                if sig % GRP == 0 and sig > 0 and sig < SIGB:
                    v.tensor_scalar(dbc[:, 0:1], lz[:, 0:1], 40.0, -40.0,
                                    AL.min, AL.max)
                    g.tensor_mul(dbcTC[:, 0:1], dbc[:, 0:1], rTC[:, 0:1])
                    g.tensor_add(beta[:, 0:1], beta[:, 0:1], dbcTC[:, 0:1])
                    v.tensor_sub(nbeta[:, 0:1], nbeta[:, 0:1], dbcTC[:, 0:1])
                    g.tensor_add(b500[:, 0:1], b500[:, 0:1], dbc[:, 0:1])
                    v.tensor_scalar_mul(dneg[:, 0:1], dbcTC[:, 0:1], -1.0)
                    s.activation(ramp[:, 0:TC + 1], iot[:, 0:TC + 1], AF.Exp,
                                 scale=dneg[:, 0:1])
                    v.tensor_mul(prv[:, 0:TC + 1], prv[:, 0:TC + 1],
                                 ramp[:, 0:TC + 1])
                    s.activation(d0row[:, 0:TC], iot[:, 0:TC], AF.Exp,
                                 scale=0.0, bias=nbeta[:, 0:1])
                    # hal_c was built with the old beta: scale by exp(dbc/TC)
                    s.activation(edb[:, 0:1], dbcTC[:, 0:1], AF.Exp)
                    g.tensor_mul(hal_c[:, 0:1], hal_c[:, 0:1], edb[:, 0:1])
                if sig < SIGB:
                    # ---- consume halo into scan init slot; hal was built
                    # without this column's renorm scale: fold rz into the copy
                    s.activation(cur[:, 0:1], hal_c[:, 0:1], AF.Copy,
                                 scale=rz[:, 0:1])
                    # ---- u = esel * rz * prv ; affine scan along t
                    u = u_pool.tile([128, TC], F32, tag="u")
                    if sig >= 1:
                        v.scalar_tensor_tensor(u[:, :], est_tiles[sig][:, :],
                                               rz[:, 0:1], prv[:, 0:TC],
                                               AL.mult, AL.mult)
                    else:
                        v.tensor_mul(u[:, :], est_tiles[sig][:, :], prv[:, 0:TC])
                    del est_tiles[sig]
                    v.tensor_tensor_scan(cur[:, 1:TC + 1], d0row[:, :], u[:, :],
                                         cur[:, 0:1], AL.mult, AL.add)
                # ---- produce halo for scan_{sig+1}: lt shifted +32 partitions
                if sig + 1 <= SIGB - 1 and sig >= 1:
                    ltsh = psum_pool.tile([128, 1], F32, tag="ltsh")
                    nc.tensor.matmul(ltsh[:, 0:1], perm[:, :], lt[:, 0:1],
                                     start=True, stop=True)
                    # bmp = beta - units(sig+1) = 2*beta - PN
                    v.tensor_sub(dneg[:, 0:1], beta[:, 0:1], PN[:, 0:1])
                    g.tensor_add(bmp[:, 0:1], dneg[:, 0:1], beta[:, 0:1])
                    # clamp: finished groups' drained columns hit the z floor
                    # and their (never-consumed) halos overflow
                    v.tensor_scalar(hs[:, 0:1], ltsh[:, 0:1], bmp[:, 0:1], 85.0,
                                    AL.add, AL.min)
                    s.activation(hal_n[:, 0:1], hs[:, 0:1], AF.Exp)
                    g.memset(hal_n[0:32, 0:1], 0.0)
                    for c in range(1, NC):
                        if 2 * c > sig + 1:
                            g.memset(hal_n[32 * c:32 * c + 32, 0:1], 0.0)
                elif sig == 0:
                    g.memset(hal_n[:, 0:1], 0.0)
                # ---- output column (lt of group 3)
                if sig >= 1 and 0 <= sig - 7 <= NS:
                    s.copy(loutT[96:128, sig - 7:sig - 6], lt[96:128, 0:1])
                # ---- birth bookkeeping
                if sig in (1, 3, 5):
                    c = (sig + 1) // 2
                    g.memset(PN[32 * c:32 * c + 32, 0:1], 0.0)
                for c in range(1, NC):
                    if sig < 2 * c:
                        g.memset(cur[32 * c:32 * c + 32, TC:TC + 1], 1.0)

        nc.sync.dma_start(out=lout_d.ap()[:, :], in_=loutT[96:128, :])

    if split_waits:
        _split_multi_waits(nc)

    _CACHE[key] = nc
    _CACHE["names"] = dict(ins=["esel", "iot", "perm"], out="lout")
    return nc, _CACHE["names"]


def host_prep(x, seqs):
    """Build per-core input arrays. Returns list of dicts."""
    f32 = np.float32
    x8 = x.reshape(NT, M, B, NF).astype(f32, copy=False)
    sq = seqs.reshape(M, B, NS)

    iot = np.broadcast_to(np.arange(512, dtype=f32), (128, 512)).copy()
    # shift-by-32 permutation: ltsh = perm.T @ lt, ltsh[q+32] = lt[q]
    perm = np.zeros((128, 128), dtype=f32)
    for q in range(96):
        perm[q, q + 32] = 1.0

    in_maps = []
    for m in range(M):
        # esel[(c,b), sig*TC + t] = E[c*TC+t, b, seqs[b, s-1]] for
        # s = sig - 2c (1 <= s <= NS), else 0.
        xm = np.ascontiguousarray(x8[:, m].transpose(1, 2, 0))   # (B, NF, NT)
        Em = np.exp(xm[:, :4] - xm[:, 4:5], dtype=f32)           # (B, 4, NT)
        idx = sq[m][:, :, None]                                   # (B, NS, 1)
        er = np.zeros((128, SIGB * TC), dtype=f32)
        erv = er.reshape(NC, B, SIGB, TC)
        for c in range(NC):
            erv[c, :, 2 * c + 1:2 * c + 1 + NS, :] = np.take_along_axis(
                Em[:, :, c * TC:(c + 1) * TC], idx, axis=1)
        in_maps.append({"esel": er, "iot": iot, "perm": perm})
    return in_maps


def host_post(x, seqlens, louts):
    f32 = np.float32
    x8 = x.reshape(NT, M, B, NF)
    C2000 = x8[:, :, :, 4].sum(axis=0, dtype=np.float32)   # (M, B)
    lt3 = np.stack([louts[m][:, 0:NS + 1] for m in range(M)])  # (M, B, NS+1)
    fwd = lt3 + C2000[:, :, None]
    fwd = fwd.reshape(NB, NS + 1)
    out = -np.take_along_axis(fwd, seqlens[:, None].astype(np.int64), axis=1) / f32(NT)
    return out.astype(np.float32)


def kernel(x, seqs, seqlens):
    nc, names = build_module()
    in_maps = host_prep(np.asarray(x), np.asarray(seqs))
    res = run_bass_kernel_spmd(nc, in_maps, list(range(M)))
    louts = [res.results[m]["lout"] for m in range(M)]
    return host_post(np.asarray(x), np.asarray(seqlens), louts)
